# revision 21
# baseline (speedup 1.0000x reference)
"""ChildSum TreeLSTM cell on 8 Trainium2 NeuronCores.

Data-parallel over the node axis N: each of the 8 cores processes N/8 nodes.
Feature dims live on SBUF partitions (2 chunks of 128 for H=256); nodes
stream along the free dim.

v4 (bf16, overhead-minimized): full-tile N=1024 matmuls (2-bank PSUM
outs, half the PE instruction count), biases folded into x's padding row
(x[300]=1, W[300]=bias*WS so activations need no bias operand), wx
re-injected into each gate PSUM via identity matmul so ACT reads gate
PSUMs directly (no DVE adds, no staging tile). h/c arrive as one fused
[128, 8, tn] tile each (single balanced DMA; wide fused elementwise).
GPSIMD takes the big sums/products, DVE the rest; outputs written as
plain per-chunk DMAs so packets spread across all 16 DMA engines.

Per 1024-node tile:
    wx     = x@Wf.T                      (PE 6 mm, DVE 2 copies)
    pg_kj  = ident@wx + h_k@Uf.T         (PE 3 mm each)
    f_kj   = sigmoid(pg_kj/WS)           (ACT from PSUM, 8 calls)
    h_tild = sum_k h_k                   (GPSIMD, 2 wide ops)
    c_tild = sum_k f_k*c_k               (GPSIMD wide mul + DVE 2 adds)
    iou    = x@Wiou.T + h_tild@Uiou.T    (PE 5 mm per gate-chunk)
    i,o,u  = sig/sig/tanh(iou/WS)        (ACT)
    c = i*u + c_tild ; h = o*tanh(c)     (DVE strided fused + ACT)
"""

import os

os.environ.setdefault("JAX_COMPILATION_CACHE_DIR", "/root/.cache/jax_bass")

import numpy as np
import ml_dtypes

import concourse.bass as bass
import concourse.mybir as mybir
import concourse.tile as tile
from concourse import bacc
from concourse.bass_utils import run_bass_kernel_spmd

BF16 = ml_dtypes.bfloat16
F32 = np.float32

N_CORES = 8
N_FULL = 65536
NSH = N_FULL // N_CORES  # nodes per core
H = 256
X_SIZE = 300
XP = 384  # x feature dim padded to 3*128 (row 300 = 1.0 carries the bias)
K = 4
TN = 1024  # nodes per on-chip tile
WS = 16.0  # weight pre-scale (exact in bf16); undone by ACT scale

SIG = mybir.ActivationFunctionType.Sigmoid
TANH = mybir.ActivationFunctionType.Tanh

LAST_RESULTS = None  # BassKernelResults of the most recent run (for test harness)


def build_bass(nsh=NSH, tn=TN):
    f32 = mybir.dt.float32
    bf = mybir.dt.bfloat16
    nt = nsh // tn
    assert nsh % tn == 0

    nc = bacc.Bacc("TRN2", debug=False)

    xt = nc.dram_tensor("xt", [3, 128, nsh], bf, kind="ExternalInput")
    ht = nc.dram_tensor("ht", [K, 2, 128, nsh], bf, kind="ExternalInput")
    ct = nc.dram_tensor("ct", [K, 2, 128, nsh], bf, kind="ExternalInput")
    wf = nc.dram_tensor("wf", [3, 128, H], bf, kind="ExternalInput")
    uf = nc.dram_tensor("uf", [2, 128, H], bf, kind="ExternalInput")
    wiou = nc.dram_tensor("wiou", [3, 128, 3 * H], bf, kind="ExternalInput")
    uiou = nc.dram_tensor("uiou", [2, 128, 3 * H], bf, kind="ExternalInput")
    # out[0] = h, out[1] = c; chunked [kind, hchunk, 128, nsh]; bf16, host upcasts
    out = nc.dram_tensor("out", [2, 2, 128, nsh], bf, kind="ExternalOutput")

    inv = 1.0 / WS

    with tile.TileContext(nc) as tc:
        with (
            tc.tile_pool(name="consts", bufs=1) as consts,
            tc.tile_pool(name="xin", bufs=2) as xin,
            tc.tile_pool(name="hin", bufs=2) as hin,
            tc.tile_pool(name="cin", bufs=2) as cin,
            tc.tile_pool(name="wxp", bufs=2) as wxp,
            tc.tile_pool(name="fpool", bufs=2) as fpool,
            tc.tile_pool(name="hsp", bufs=1) as hsp,
            tc.tile_pool(name="htp", bufs=2) as htp,
            tc.tile_pool(name="ctp", bufs=2) as ctp,
            tc.tile_pool(name="gio", bufs=2) as gio,
            tc.tile_pool(name="scr", bufs=1) as scr,
            tc.tile_pool(name="outp", bufs=2) as outp,
            tc.tile_pool(name="psA", bufs=2, space="PSUM") as psA,
        ):

            def mm(out2, lhsT, rhs2, start, stop):
                # matmul with [*, tn] operands split into 512-col halves
                # (a PSUM matmul write must stay within one 2KB bank)
                for hh in range(tn // 512):
                    ssl = slice(hh * 512, (hh + 1) * 512)
                    nc.tensor.matmul(
                        out2[:, ssl], lhsT, rhs2[:, ssl], start=start, stop=stop
                    )
            # gate-critical consts first so the first wx/gate matmuls start early
            wf_s = consts.tile([128, 3, H], bf)
            nc.sync.dma_start(wf_s[:], wf[:].rearrange("c p m -> p c m"))
            uf_s = consts.tile([128, 2, H], bf)
            nc.sync.dma_start(uf_s[:], uf[:].rearrange("c p m -> p c m"))
            wiou_s = consts.tile([128, 3, 3 * H], bf)
            nc.sync.dma_start(wiou_s[:], wiou[:].rearrange("c p m -> p c m"))
            uiou_s = consts.tile([128, 2, 3 * H], bf)
            nc.sync.dma_start(uiou_s[:], uiou[:].rearrange("c p m -> p c m"))

            for t in range(nt):
                n0 = t * tn
                nsl = slice(n0, n0 + tn)

                xtile = xin.tile([128, 3, tn], bf, tag="x")
                nc.sync.dma_start(
                    xtile[:], xt[:, :, nsl].rearrange("c p n -> p c n")
                )
                # fused h/c tiles: slot (k, chunk j) at dim1 = 2k + j
                hbig = hin.tile([128, 2 * K, tn], bf, tag="h")
                nc.sync.dma_start(
                    hbig[:], ht[:, :, :, nsl].rearrange("k c p n -> p (k c) n")
                )
                cbig = cin.tile([128, 2 * K, tn], bf, tag="c")
                nc.sync.dma_start(
                    cbig[:], ct[:, :, :, nsl].rearrange("k c p n -> p (k c) n")
                )

                # h_tild = sum_k h_k  (GPSIMD, 2 wide ops)
                hsum = hsp.tile([128, 4, tn], bf, tag="hs")
                nc.gpsimd.tensor_add(hsum[:], hbig[:, 0:4, :], hbig[:, 4:8, :])
                htild = htp.tile([128, 2, tn], bf, tag="htild")
                nc.gpsimd.tensor_add(htild[:], hsum[:, 0:2, :], hsum[:, 2:4, :])

                # wx = x@Wf.T (scaled), both chunks in one 4-bank PSUM tile
                pwx = psA.tile([128, 2, tn], f32, tag="ps")
                for j in range(2):
                    jsl = slice(j * 128, (j + 1) * 128)
                    for xc in range(3):
                        mm(
                            pwx[:, j, :],
                            wf_s[:, xc, jsl],
                            xtile[:, xc, :],
                            start=(xc == 0),
                            stop=(xc == 2),
                        )
                wx_sb = wxp.tile([128, 2, tn], bf, tag="wx")
                nc.vector.tensor_copy(wx_sb[:], pwx[:])

                # forget gates: uh_k per chunk; fused DVE add of wx; one
                # batched in-place sigmoid for the whole tile afterwards
                fin = fpool.tile([128, 2 * K, tn], bf, tag="f")
                f_t = fin
                for k in range(K):
                    pg = psA.tile([128, 2, tn], f32, tag="ps")
                    for j in range(2):
                        jsl = slice(j * 128, (j + 1) * 128)
                        for hc in range(2):
                            mm(
                                pg[:, j, :],
                                uf_s[:, hc, jsl],
                                hbig[:, 2 * k + hc, :],
                                start=(hc == 0),
                                stop=(hc == 1),
                            )
                    nc.vector.tensor_add(
                        fin[:, 2 * k : 2 * k + 2, :], pg[:], wx_sb[:]
                    )
                nc.scalar.activation(f_t[:], fin[:], SIG, scale=inv)

                # c_tild = sum_k f_k * c_k (GPSIMD wide mul + DVE adds)
                nc.gpsimd.tensor_mul(f_t[:], f_t[:], cbig[:])
                s03 = hsp.tile([128, 4, tn], bf, tag="hs")
                nc.vector.tensor_add(s03[:], f_t[:, 0:4, :], f_t[:, 4:8, :])
                ctild = ctp.tile([128, 2, tn], bf, tag="ctild")
                nc.vector.tensor_add(ctild[:], s03[:, 0:2, :], s03[:, 2:4, :])

                # iou matmuls (x-side + h_tild-side)
                def iou_mms(pdst, oc):
                    osl = slice(oc * 128, (oc + 1) * 128)
                    for xc in range(3):
                        mm(
                            pdst,
                            wiou_s[:, xc, osl],
                            xtile[:, xc, :],
                            start=(xc == 0),
                            stop=False,
                        )
                    for hc in range(2):
                        mm(
                            pdst,
                            uiou_s[:, hc, osl],
                            htild[:, hc, :],
                            start=False,
                            stop=(hc == 1),
                        )

                # i,o batched per chunk in a 4-bank PSUM pair; u both chunks
                g_io2 = gio.tile([128, 4, tn], bf, tag="gio")  # [i0,o0,i1,o1]
                for j in range(2):
                    pio = psA.tile([128, 2, tn], f32, tag="ps")
                    iou_mms(pio[:, 0, :], j)          # i (chunk j)
                    iou_mms(pio[:, 1, :], 2 + j)      # o (chunk j)
                    nc.scalar.activation(
                        g_io2[:, 2 * j : 2 * j + 2, :], pio[:], SIG, scale=inv
                    )
                g_u = gio.tile([128, 2, tn], bf, tag="gu")
                pu = psA.tile([128, 2, tn], f32, tag="ps")
                iou_mms(pu[:, 0, :], 4)               # u chunk 0
                iou_mms(pu[:, 1, :], 5)               # u chunk 1
                nc.scalar.activation(g_u[:], pu[:], TANH, scale=inv)

                # outputs: c = i*u + ctild; h = o*tanh(c)  (strided fused DVE)
                ciu = scr.tile([128, 2, tn], bf, tag="ciu")
                nc.vector.tensor_mul(ciu[:], g_io2[:, 0:4:2, :], g_u[:])
                c_t = outp.tile([128, 2, tn], bf, tag="cout")
                nc.vector.tensor_add(c_t[:], ciu[:], ctild[:])
                tanh_c = scr.tile([128, 2, tn], bf, tag="tanhc")
                nc.scalar.activation(tanh_c[:], c_t[:], TANH)
                h_t = outp.tile([128, 2, tn], bf, tag="hout")
                nc.vector.tensor_mul(h_t[:], g_io2[:, 1:4:2, :], tanh_c[:])

                # plain per-chunk output DMAs (keeps all 16 DMA engines busy)
                for j in range(2):
                    nc.sync.dma_start(out[0, j, :, nsl], h_t[:, j, :])
                    nc.sync.dma_start(out[1, j, :, nsl], c_t[:, j, :])

    nc.compile()
    return nc


_NC_CACHE = {}


def _get_nc(nsh, tn):
    key = (nsh, tn)
    if key not in _NC_CACHE:
        _NC_CACHE[key] = build_bass(nsh, tn)
    return _NC_CACHE[key]


def prep_host_inputs(x, h_msgs, c_msgs, W_iou, b_iou, U_iou, b_Uiou, W_f, b_Wf, U_f, b_Uf):
    """Full-input -> per-core input maps (host-side layout only)."""
    n = x.shape[0]
    nsh = n // N_CORES

    xp = np.zeros((XP, n), F32)
    xp[:X_SIZE] = x.T
    xp[X_SIZE] = 1.0  # constant row carrying the bias through the matmul
    xt_full = np.ascontiguousarray(xp).astype(BF16).reshape(3, 128, n)

    ht_full = np.ascontiguousarray(h_msgs.astype(BF16).transpose(1, 2, 0)).reshape(
        K, 2, 128, n
    )
    ct_full = np.ascontiguousarray(c_msgs.astype(BF16).transpose(1, 2, 0)).reshape(
        K, 2, 128, n
    )

    wfp = np.zeros((XP, H), F32)
    wfp[:X_SIZE] = W_f.T * WS
    wfp[X_SIZE] = (b_Wf + b_Uf) * WS
    wf_host = wfp.astype(BF16).reshape(3, 128, H)
    uf_host = np.ascontiguousarray(U_f.T * WS).astype(BF16).reshape(2, 128, H)
    wioup = np.zeros((XP, 3 * H), F32)
    wioup[:X_SIZE] = W_iou.T * WS
    wioup[X_SIZE] = (b_iou + b_Uiou) * WS
    wiou_host = wioup.astype(BF16).reshape(3, 128, 3 * H)
    uiou_host = np.ascontiguousarray(U_iou.T * WS).astype(BF16).reshape(2, 128, 3 * H)

    in_maps = []
    for c in range(N_CORES):
        sl = slice(c * nsh, (c + 1) * nsh)
        in_maps.append(
            {
                "xt": np.ascontiguousarray(xt_full[:, :, sl]),
                "ht": np.ascontiguousarray(ht_full[:, :, :, sl]),
                "ct": np.ascontiguousarray(ct_full[:, :, :, sl]),
                "wf": wf_host,
                "uf": uf_host,
                "wiou": wiou_host,
                "uiou": uiou_host,
            }
        )
    return in_maps


def kernel(**inputs):
    global LAST_RESULTS
    inputs = {k: np.asarray(v) for k, v in inputs.items()}
    n = inputs["x"].shape[0]
    assert n == N_FULL, f"hardcoded for N={N_FULL}, got {n}"
    nsh = n // N_CORES

    nc = _get_nc(nsh, TN)
    in_maps = prep_host_inputs(**inputs)

    res = None
    for attempt in range(3):
        try:
            res = run_bass_kernel_spmd(nc, in_maps, core_ids=list(range(N_CORES)))
            break
        except Exception:
            if attempt == 2:
                raise
            import time as _time

            _time.sleep(5.0)
    LAST_RESULTS = res

    # results[c]["out"]: [2, 2, 128, nsh] -> full [2, N, 256]
    per_core = [r["out"].astype(F32).reshape(2, 256, nsh) for r in res.results]
    full = np.concatenate(per_core, axis=-1)  # [2, 256, N]
    return np.ascontiguousarray(full.transpose(0, 2, 1)).astype(F32)


# revision 22
# speedup vs baseline: 1.2159x; 1.2159x over previous
"""ChildSum TreeLSTM cell on 8 Trainium2 NeuronCores.

Data-parallel over the node axis N: each of the 8 cores processes N/8 nodes.
Feature dims live on SBUF partitions (2 chunks of 128 for H=256); nodes
stream along the free dim.

v5 (bf16, 2D-contiguous APs): all elementwise tiles are flat [128, M]
with slice arithmetic — multi-dim APs fall off the DVE/GPSIMD fast path
(measured 3-6x slower). Biases are folded into x's padding row (x[300]=1,
W[300]=bias*WS) so activations need no bias operand and batch per tile;
WS=16 weight pre-scale is undone for free via the ACT `scale` field.
wx is added to the gate PSUMs by DVE (no identity matmuls). h/c arrive
as one fused DMA each; i/o/u live in separate contiguous tiles so the
output stage is 3 flat DVE ops. Outputs are plain per-chunk DMAs so
packets spread across all 16 DMA engines.

Per 1024-node tile (layout slots: k-major (k, chunk) for f/h/c):
    wx     = x@Wf.T                    (PE 12 mm)
    fin_k  = h_k@Uf.T + wx             (PE 8 mm/k, DVE 1 add/k)
    f      = sigmoid(fin/WS)           (ACT, 1 call)
    h_tild = sum_k h_k                 (GPSIMD, 2 flat ops)
    c_tild = sum_k f_k*c_k             (DVE: flat mul + 2 adds)
    iou    = x@Wiou.T + h_tild@Uiou.T  (PE 10 mm/gate)
    i,o,u  = sig/sig/tanh(iou/WS)      (ACT, 3 calls)
    c = i*u + c_tild ; h = o*tanh(c)   (DVE 3 flat ops + ACT)
"""

import os

os.environ.setdefault("JAX_COMPILATION_CACHE_DIR", "/root/.cache/jax_bass")

import numpy as np
import ml_dtypes

import concourse.bass as bass
import concourse.mybir as mybir
import concourse.tile as tile
from concourse import bacc
from concourse.bass_utils import run_bass_kernel_spmd

BF16 = ml_dtypes.bfloat16
F32 = np.float32

N_CORES = 8
N_FULL = 65536
NSH = N_FULL // N_CORES  # nodes per core
H = 256
X_SIZE = 300
XP = 384  # x feature dim padded to 3*128 (row 300 = 1.0 carries the bias)
K = 4
TN = 1024  # nodes per on-chip tile
WS = 16.0  # weight pre-scale (exact in bf16); undone by ACT scale

SIG = mybir.ActivationFunctionType.Sigmoid
TANH = mybir.ActivationFunctionType.Tanh

LAST_RESULTS = None  # BassKernelResults of the most recent run (for test harness)


def build_bass(nsh=NSH, tn=TN):
    f32 = mybir.dt.float32
    bf = mybir.dt.bfloat16
    nt = nsh // tn
    assert nsh % tn == 0

    nc = bacc.Bacc("TRN2", debug=False)

    xt = nc.dram_tensor("xt", [3, 128, nsh], bf, kind="ExternalInput")
    ht = nc.dram_tensor("ht", [K, 2, 128, nsh], bf, kind="ExternalInput")
    ct = nc.dram_tensor("ct", [K, 2, 128, nsh], bf, kind="ExternalInput")
    wf = nc.dram_tensor("wf", [3, 128, H], bf, kind="ExternalInput")
    uf = nc.dram_tensor("uf", [2, 128, H], bf, kind="ExternalInput")
    wiou = nc.dram_tensor("wiou", [3, 128, 3 * H], bf, kind="ExternalInput")
    uiou = nc.dram_tensor("uiou", [2, 128, 3 * H], bf, kind="ExternalInput")
    # out[0] = h, out[1] = c; chunked [kind, hchunk, 128, nsh]; bf16, host upcasts
    out = nc.dram_tensor("out", [2, 2, 128, nsh], bf, kind="ExternalOutput")

    inv = 1.0 / WS

    with tile.TileContext(nc) as tc:
        with (
            tc.tile_pool(name="consts", bufs=1) as consts,
            tc.tile_pool(name="xin", bufs=2) as xin,
            tc.tile_pool(name="hin", bufs=2) as hin,
            tc.tile_pool(name="cin", bufs=2) as cin,
            tc.tile_pool(name="wxp", bufs=2) as wxp,
            tc.tile_pool(name="fpool", bufs=2) as fpool,
            tc.tile_pool(name="hsp", bufs=1) as hsp,
            tc.tile_pool(name="htp", bufs=2) as htp,
            tc.tile_pool(name="ctp", bufs=2) as ctp,
            tc.tile_pool(name="gio", bufs=2) as gio,
            tc.tile_pool(name="scr", bufs=1) as scr,
            tc.tile_pool(name="outp", bufs=2) as outp,
            tc.tile_pool(name="psA", bufs=2, space="PSUM") as psA,
        ):

            def mm(out2, lhsT, rhs2, start, stop):
                # matmul with [*, tn] operands split into 512-col halves
                # (a PSUM matmul write must stay within one 2KB bank)
                for hh in range(tn // 512):
                    ssl = slice(hh * 512, (hh + 1) * 512)
                    nc.tensor.matmul(
                        out2[:, ssl], lhsT, rhs2[:, ssl], start=start, stop=stop
                    )

            # gate-critical consts first so the first wx/gate matmuls start early
            wf_s = consts.tile([128, 3, H], bf)
            nc.sync.dma_start(wf_s[:], wf[:].rearrange("c p m -> p c m"))
            uf_s = consts.tile([128, 2, H], bf)
            nc.sync.dma_start(uf_s[:], uf[:].rearrange("c p m -> p c m"))
            wiou_s = consts.tile([128, 3, 3 * H], bf)
            nc.sync.dma_start(wiou_s[:], wiou[:].rearrange("c p m -> p c m"))
            uiou_s = consts.tile([128, 2, 3 * H], bf)
            nc.sync.dma_start(uiou_s[:], uiou[:].rearrange("c p m -> p c m"))

            for t in range(nt):
                n0 = t * tn
                nsl = slice(n0, n0 + tn)

                xtile = xin.tile([128, 3 * tn], bf, tag="x")
                nc.sync.dma_start(
                    xtile[:], xt[:, :, nsl].rearrange("c p n -> p c n")
                )
                # fused h/c tiles: flat [128, 8*tn]; slot (k, chunk j) = 2k+j
                hbig = hin.tile([128, 2 * K * tn], bf, tag="h")
                nc.sync.dma_start(
                    hbig[:], ht[:, :, :, nsl].rearrange("k c p n -> p (k c) n")
                )
                cbig = cin.tile([128, 2 * K * tn], bf, tag="c")
                nc.sync.dma_start(
                    cbig[:], ct[:, :, :, nsl].rearrange("k c p n -> p (k c) n")
                )

                # h_tild = sum_k h_k  (GPSIMD, flat 2D ops)
                hsum = hsp.tile([128, 4 * tn], bf, tag="hs")
                nc.gpsimd.tensor_add(
                    hsum[:], hbig[:, : 4 * tn], hbig[:, 4 * tn :]
                )
                htild = htp.tile([128, 2 * tn], bf, tag="htild")
                nc.gpsimd.tensor_add(
                    htild[:], hsum[:, : 2 * tn], hsum[:, 2 * tn :]
                )

                # wx = x@Wf.T (scaled), both chunks in one 4-bank PSUM tile
                pwx = psA.tile([128, 2 * tn], f32, tag="ps")
                for j in range(2):
                    jsl = slice(j * 128, (j + 1) * 128)
                    for xc in range(3):
                        mm(
                            pwx[:, j * tn : (j + 1) * tn],
                            wf_s[:, xc, jsl],
                            xtile[:, xc * tn : (xc + 1) * tn],
                            start=(xc == 0),
                            stop=(xc == 2),
                        )
                wx_sb = wxp.tile([128, 2 * tn], bf, tag="wx")
                nc.vector.tensor_copy(wx_sb[:], pwx[:])

                # forget gates: uh_k both chunks per PSUM tile; DVE adds wx;
                # one batched in-place sigmoid for the whole tile afterwards
                fin = fpool.tile([128, 2 * K * tn], bf, tag="f")
                f_t = fin
                for k in range(K):
                    pg = psA.tile([128, 2 * tn], f32, tag="ps")
                    for j in range(2):
                        jsl = slice(j * 128, (j + 1) * 128)
                        for hc in range(2):
                            mm(
                                pg[:, j * tn : (j + 1) * tn],
                                uf_s[:, hc, jsl],
                                hbig[:, (2 * k + hc) * tn : (2 * k + hc + 1) * tn],
                                start=(hc == 0),
                                stop=(hc == 1),
                            )
                    nc.vector.tensor_add(
                        fin[:, 2 * k * tn : (2 * k + 2) * tn], pg[:], wx_sb[:]
                    )
                nc.scalar.activation(f_t[:], fin[:], SIG, scale=inv)

                # c_tild = sum_k f_k * c_k (flat DVE mul + adds)
                nc.vector.tensor_mul(f_t[:], f_t[:], cbig[:])
                s03 = hsp.tile([128, 4 * tn], bf, tag="hs")
                nc.vector.tensor_add(s03[:], f_t[:, : 4 * tn], f_t[:, 4 * tn :])
                ctild = ctp.tile([128, 2 * tn], bf, tag="ctild")
                nc.vector.tensor_add(ctild[:], s03[:, : 2 * tn], s03[:, 2 * tn :])

                # iou matmuls (x-side + h_tild-side)
                def iou_mms(pdst, oc):
                    osl = slice(oc * 128, (oc + 1) * 128)
                    for xc in range(3):
                        mm(
                            pdst,
                            wiou_s[:, xc, osl],
                            xtile[:, xc * tn : (xc + 1) * tn],
                            start=(xc == 0),
                            stop=False,
                        )
                    for hc in range(2):
                        mm(
                            pdst,
                            uiou_s[:, hc, osl],
                            htild[:, hc * tn : (hc + 1) * tn],
                            start=False,
                            stop=(hc == 1),
                        )

                # i / o / u each in their own [c0,c1] PSUM tile -> flat SBUF
                gates = {}
                for name, base, func in (("i", 0, SIG), ("o", 2, SIG), ("u", 4, TANH)):
                    pio = psA.tile([128, 2 * tn], f32, tag="ps")
                    iou_mms(pio[:, :tn], base)           # chunk 0
                    iou_mms(pio[:, tn:], base + 1)       # chunk 1
                    g = gio.tile([128, 2 * tn], bf, tag="g" + name)
                    nc.scalar.activation(g[:], pio[:], func, scale=inv)
                    gates[name] = g

                # outputs: c = i*u + ctild; h = o*tanh(c)  (flat DVE)
                ciu = scr.tile([128, 2 * tn], bf, tag="ciu")
                nc.vector.tensor_mul(ciu[:], gates["i"][:], gates["u"][:])
                c_t = outp.tile([128, 2 * tn], bf, tag="cout")
                nc.vector.tensor_add(c_t[:], ciu[:], ctild[:])
                tanh_c = scr.tile([128, 2 * tn], bf, tag="tanhc")
                nc.scalar.activation(tanh_c[:], c_t[:], TANH)
                h_t = outp.tile([128, 2 * tn], bf, tag="hout")
                nc.vector.tensor_mul(h_t[:], gates["o"][:], tanh_c[:])

                # plain per-chunk output DMAs (keeps all 16 DMA engines busy)
                for j in range(2):
                    nc.sync.dma_start(
                        out[0, j, :, nsl], h_t[:, j * tn : (j + 1) * tn]
                    )
                    nc.sync.dma_start(
                        out[1, j, :, nsl], c_t[:, j * tn : (j + 1) * tn]
                    )

    nc.compile()
    return nc


_NC_CACHE = {}


def _get_nc(nsh, tn):
    key = (nsh, tn)
    if key not in _NC_CACHE:
        _NC_CACHE[key] = build_bass(nsh, tn)
    return _NC_CACHE[key]


def prep_host_inputs(x, h_msgs, c_msgs, W_iou, b_iou, U_iou, b_Uiou, W_f, b_Wf, U_f, b_Uf):
    """Full-input -> per-core input maps (host-side layout only)."""
    n = x.shape[0]
    nsh = n // N_CORES

    xp = np.zeros((XP, n), F32)
    xp[:X_SIZE] = x.T
    xp[X_SIZE] = 1.0  # constant row carrying the bias through the matmul
    xt_full = np.ascontiguousarray(xp).astype(BF16).reshape(3, 128, n)

    ht_full = np.ascontiguousarray(h_msgs.astype(BF16).transpose(1, 2, 0)).reshape(
        K, 2, 128, n
    )
    ct_full = np.ascontiguousarray(c_msgs.astype(BF16).transpose(1, 2, 0)).reshape(
        K, 2, 128, n
    )

    wfp = np.zeros((XP, H), F32)
    wfp[:X_SIZE] = W_f.T * WS
    wfp[X_SIZE] = (b_Wf + b_Uf) * WS
    wf_host = wfp.astype(BF16).reshape(3, 128, H)
    uf_host = np.ascontiguousarray(U_f.T * WS).astype(BF16).reshape(2, 128, H)
    wioup = np.zeros((XP, 3 * H), F32)
    wioup[:X_SIZE] = W_iou.T * WS
    wioup[X_SIZE] = (b_iou + b_Uiou) * WS
    wiou_host = wioup.astype(BF16).reshape(3, 128, 3 * H)
    uiou_host = np.ascontiguousarray(U_iou.T * WS).astype(BF16).reshape(2, 128, 3 * H)

    in_maps = []
    for c in range(N_CORES):
        sl = slice(c * nsh, (c + 1) * nsh)
        in_maps.append(
            {
                "xt": np.ascontiguousarray(xt_full[:, :, sl]),
                "ht": np.ascontiguousarray(ht_full[:, :, :, sl]),
                "ct": np.ascontiguousarray(ct_full[:, :, :, sl]),
                "wf": wf_host,
                "uf": uf_host,
                "wiou": wiou_host,
                "uiou": uiou_host,
            }
        )
    return in_maps


def kernel(**inputs):
    global LAST_RESULTS
    inputs = {k: np.asarray(v) for k, v in inputs.items()}
    n = inputs["x"].shape[0]
    assert n == N_FULL, f"hardcoded for N={N_FULL}, got {n}"
    nsh = n // N_CORES

    nc = _get_nc(nsh, TN)
    in_maps = prep_host_inputs(**inputs)

    res = None
    for attempt in range(3):
        try:
            res = run_bass_kernel_spmd(nc, in_maps, core_ids=list(range(N_CORES)))
            break
        except Exception:
            if attempt == 2:
                raise
            import time as _time

            _time.sleep(5.0)
    LAST_RESULTS = res

    # results[c]["out"]: [2, 2, 128, nsh] -> full [2, N, 256]
    per_core = [r["out"].astype(F32).reshape(2, 256, nsh) for r in res.results]
    full = np.concatenate(per_core, axis=-1)  # [2, 256, N]
    return np.ascontiguousarray(full.transpose(0, 2, 1)).astype(F32)
